# revision 1
# baseline (speedup 1.0000x reference)
"""MoE (top-2 of 8 experts) Trainium2 kernel across 8 NeuronCores.

Host side: gating matmul + top-2 + softmax + token dispatch (part of input
sharding). Device side (per core, SPMD): TWO half-expert FFN shards. Each
expert's FFN is split into two H-halves placed on two cores; the per-core
"slot 0" runs a half of one of the 4 most-loaded (hot) experts and "slot 1"
a half of one of the 4 least-loaded (cold) experts, so the fixed SPMD token
capacities are (max hot load, max cold load) instead of (max load) twice —
near-perfect load balance. The two partial y's of an expert are summed on
the host during unsharding (b2, which is 0 here, would also be added there).

Per slot, in transposed layout:
    hT = gelu(w1h.T @ xeT + b1h)       [H/2, C_s]
    yT = (w2h.T @ hT) * wc             [D, C_s]   (partial over its H-half)

Matmuls run in fp8 (e4m3) DoubleRow perf mode with 3-term error
compensation: every operand T is split T = T_hi + T_lo (both e4m3, lo is
the quantization residual), and each logical product computes
    W_hi@x_hi + W_lo@x_hi + W_hi@x_lo
accumulated in fp32 PSUM (the dropped W_lo@x_lo term is ~0.06% relative).
This carries ~8 significant bits per operand (bf16-level accuracy) at 0.75x
the PE cost of bf16 in DoubleRow mode (2 k-tiles per instruction at 0.5
cycles/row). Weights are pre-scaled by SW=1024 so w-values sit in e4m3's
normal range; the descale is fused into the gelu activation (mm1) and the
host-prescaled combine weights (mm2).

All DRAM layouts are chunk-/block-major so every DMA moves >=512B
contiguous innermost runs (smaller elements pay a 2x DMA penalty); x blocks
are zero-padded to TB columns (the DMA moves the pad, matmuls never read
it).
"""

import numpy as np
import ml_dtypes

import jax
from jax.experimental.shard_map import shard_map
from jax.sharding import Mesh, PartitionSpec

import concourse.bass as bass  # noqa: F401
import concourse.mybir as mybir
import concourse.tile as tile
from concourse import bacc
from concourse.bass2jax import (
    _bass_exec_p,
    install_neuronx_cc_hook,
    partition_id_tensor,
)

B, S, D, H, E, TOPK = 4, 2048, 1024, 4096, 8, 2
T = B * S
P = 128
TB = 512  # max token block (matmul free dim)
SW = 1024.0  # weight pre-scale so w-values sit in e4m3 normal range
H2 = H // 2  # per-slot H half

DK = D // P  # 8  k-tiles for mm1 (contraction over D)
HK2 = H2 // P  # 16 k-tiles for mm2 (contraction over H/2), mm1 out tiles

CW = 512  # weight chunk columns

_F8 = mybir.dt.float8e4
_BF16 = mybir.dt.bfloat16
_F32 = mybir.dt.float32
_E4 = ml_dtypes.float8_e4m3
_DR = mybir.MatmulPerfMode.DoubleRow

# cache of built+compiled runners keyed by capacities, so repeat calls with
# the same shapes reuse the in-process jit executable (no re-trace/re-compile)
_RUNNER_CACHE: dict[tuple, tuple] = {}


def _make_runner(nc, n_cores=8):
    """Persistent jitted SPMD runner for a Bass module (mirrors
    concourse.bass2jax.run_bass_via_pjrt, but reusable across calls)."""
    install_neuronx_cc_hook()
    partition_name = nc.partition_id_tensor.name if nc.partition_id_tensor else None
    in_names, out_names, out_avals, zero_outs = [], [], [], []
    for alloc in nc.m.functions[0].allocations:
        if not isinstance(alloc, mybir.MemoryLocationSet):
            continue
        name = alloc.memorylocations[0].name
        if alloc.kind == "ExternalInput":
            if name != partition_name:
                in_names.append(name)
        elif alloc.kind == "ExternalOutput":
            out_names.append(name)
            shape = tuple(alloc.tensor_shape)
            dtype = mybir.dt.np(alloc.dtype)
            out_avals.append(jax.core.ShapedArray(shape, dtype))
            zero_outs.append(np.zeros(shape, dtype))
    n_params = len(in_names)
    all_in_names = list(in_names) + list(out_names)
    if partition_name is not None:
        all_in_names.append(partition_name)

    def _body(*args):
        operands = list(args)
        if partition_name is not None:
            operands.append(partition_id_tensor())
        outs = _bass_exec_p.bind(
            *operands,
            out_avals=tuple(out_avals),
            in_names=tuple(all_in_names),
            out_names=tuple(out_names),
            lowering_input_output_aliases=(),
            sim_require_finite=True,
            sim_require_nnan=True,
            nc=nc,
        )
        return tuple(outs)

    devices = jax.devices()[:n_cores]
    mesh = Mesh(np.asarray(devices), ("core",))
    n_outs = len(out_avals)
    in_specs = (PartitionSpec("core"),) * (n_params + n_outs)
    out_specs = (PartitionSpec("core"),) * n_outs
    f = jax.jit(
        shard_map(
            _body, mesh=mesh, in_specs=in_specs, out_specs=out_specs, check_rep=False
        ),
        donate_argnums=tuple(range(n_params, n_params + n_outs)),
        keep_unused=True,
    )

    def run(in_maps):
        concat_in = [
            np.concatenate([np.asarray(m[name]) for m in in_maps], axis=0)
            for name in in_names
        ]
        concat_zeros = [
            np.zeros((n_cores * z.shape[0], *z.shape[1:]), z.dtype) for z in zero_outs
        ]
        outs = f(*concat_in, *concat_zeros)
        return [
            {
                name: np.asarray(outs[i]).reshape(n_cores, *out_avals[i].shape)[c]
                for i, name in enumerate(out_names)
            }
            for c in range(n_cores)
        ]

    return run


def _blocks(C, thin_tail=0):
    """Split C columns into even blocks of at most TB (plus an optional thin
    final block, which shortens the end-of-kernel epilogue chain)."""
    if thin_tail and C > TB:
        starts, sizes = _blocks(C - thin_tail)
        return starts + [C - thin_tail], sizes + [thin_tail]
    n_blocks = (C + TB - 1) // TB
    base, rem = divmod(C, n_blocks)
    sizes = [base + (1 if i < rem else 0) for i in range(n_blocks)]
    starts = [sum(sizes[:i]) for i in range(n_blocks)]
    return starts, sizes


def _build(C0: int, C1: int):
    """Bass module: slot0 = half-FFN over C0 tokens, slot1 over C1 tokens."""
    nc = bacc.Bacc("TRN2", target_bir_lowering=False, debug=False, num_devices=8)

    NC1 = H2 // CW  # 4 w1 chunks per slot
    NC2 = D // CW  # 2 w2 chunks per slot
    caps = [C0, C1]
    col0 = [0, C0]  # column offset of each slot in wc / yT
    sblocks = [_blocks(C0), _blocks(C1)]
    NB = [len(sblocks[0][1]), len(sblocks[1][1])]
    # global block list: (slot, ib, dram_index, col_start, width)
    gblocks = []
    for s in (0, 1):
        for ib in range(NB[s]):
            gblocks.append(
                (s, ib, len(gblocks), col0[s] + sblocks[s][0][ib], sblocks[s][1][ib])
            )
    NBT = len(gblocks)
    CT = C0 + C1

    xhi = nc.dram_tensor("xhi", [P, NBT, DK, TB], _F8, kind="ExternalInput")
    xlo = nc.dram_tensor("xlo", [P, NBT, DK, TB], _F8, kind="ExternalInput")
    w1hi = nc.dram_tensor("w1hi", [P, 2, NC1, DK, CW], _F8, kind="ExternalInput")
    w1lo = nc.dram_tensor("w1lo", [P, 2, NC1, DK, CW], _F8, kind="ExternalInput")
    w2hi = nc.dram_tensor("w2hi", [P, 2, NC2, HK2, CW], _F8, kind="ExternalInput")
    w2lo = nc.dram_tensor("w2lo", [P, 2, NC2, HK2, CW], _F8, kind="ExternalInput")
    b1t = nc.dram_tensor("b1t", [P, 2, HK2], _F32, kind="ExternalInput")
    wc = nc.dram_tensor("wc", [P, CT], _BF16, kind="ExternalInput")  # pre /SW
    yT = nc.dram_tensor("yT", [D, CT], _BF16, kind="ExternalOutput")

    with tile.TileContext(nc) as tc:
        with (
            tc.tile_pool(name="wpool", bufs=1) as wpool,
            tc.tile_pool(name="xpool", bufs=2) as xpool,
            tc.tile_pool(name="hpool", bufs=2) as hpool,
            tc.tile_pool(name="h32pool", bufs=4) as h32pool,
            tc.tile_pool(name="ypool", bufs=3) as ypool,
            tc.tile_pool(name="psum1", bufs=4, space="PSUM") as psum1,
            tc.tile_pool(name="psum2", bufs=4, space="PSUM") as psum2,
        ):
            x_tiles = {}

            def load_x(g, split=False):
                xh = xpool.tile([P, DK, TB], _F8, name="xh_sb")
                xl = xpool.tile([P, DK, TB], _F8, name="xl_sb")
                if split:
                    # two half-k DMAs so the first matmul group can start
                    # after only half of x0_hi has landed
                    nc.sync.dma_start(xh[:, : DK // 2, :], xhi[:, g, : DK // 2, :])
                    nc.sync.dma_start(xh[:, DK // 2 :, :], xhi[:, g, DK // 2 :, :])
                else:
                    nc.sync.dma_start(xh[:, :, :], xhi[:, g, :, :])
                    nc.sync.dma_start(xl[:, :, :], xlo[:, g, :, :])
                x_tiles[g] = (xh, xl)
                return (xh, xl)

            w1hi_sb = [
                [wpool.tile([P, DK, CW], _F8, name=f"w1hi{s}_{j}") for j in range(NC1)]
                for s in (0, 1)
            ]
            w1lo_sb = [
                [wpool.tile([P, DK, CW], _F8, name=f"w1lo{s}_{j}") for j in range(NC1)]
                for s in (0, 1)
            ]
            w2hi_sb = [
                [wpool.tile([P, HK2, CW], _F8, name=f"w2hi{s}_{j}") for j in range(NC2)]
                for s in (0, 1)
            ]
            w2lo_sb = [
                [wpool.tile([P, HK2, CW], _F8, name=f"w2lo{s}_{j}") for j in range(NC2)]
                for s in (0, 1)
            ]
            b1_sb = wpool.tile([P, 2, HK2], _F32, name="b1_sb")
            wc_sb = wpool.tile([P, CT], _BF16, name="wc_sb")

            def load_w1(s, j):
                nc.sync.dma_start(w1hi_sb[s][j][:], w1hi[:, s, j, :, :])
                nc.sync.dma_start(w1lo_sb[s][j][:], w1lo[:, s, j, :, :])

            def load_w2(s, j):
                nc.sync.dma_start(w2hi_sb[s][j][:], w2hi[:, s, j, :, :])
                nc.sync.dma_start(w2lo_sb[s][j][:], w2lo[:, s, j, :, :])

            # startup order mirrors first consumption: the PE's first term
            # group needs only w1hi chunk0 cols 0:128 + x0_hi k0..3, the
            # second adds w1lo, the third x0_lo; b1 unblocks the first gelu
            # (which frees PSUM buffers)
            nc.sync.dma_start(w1hi_sb[0][0][:, :, 0:P], w1hi[:, 0, 0, :, 0:P])
            xh0, xl0 = load_x(0, split=True)
            nc.sync.dma_start(
                w1hi_sb[0][0][:, :, P:CW], w1hi[:, 0, 0, :, P:CW]
            )
            nc.sync.dma_start(w1lo_sb[0][0][:], w1lo[:, 0, 0, :, :])
            nc.sync.dma_start(xl0[:, : DK // 2, :], xlo[:, 0, : DK // 2, :])
            nc.sync.dma_start(xl0[:, DK // 2 :, :], xlo[:, 0, DK // 2 :, :])
            nc.sync.dma_start(b1_sb[:], b1t[:])
            for j in range(1, NC1):
                load_w1(0, j)
            nc.sync.dma_start(wc_sb[:], wc[:])
            for j in range(NC2):
                load_w2(0, j)
            if NBT > 1:
                load_x(1)
            for j in range(NC1):
                load_w1(1, j)
            for j in range(NC2):
                load_w2(1, j)

            for s, ib, g, t0, tw in gblocks:
                if g not in x_tiles:
                    load_x(g)
                xh, xl = x_tiles.pop(g)
                if g + 2 < NBT:
                    load_x(g + 2)  # blocks 0 and 1 were loaded up front

                h_hi = hpool.tile([P, HK2, TB], _F8, name="hhi_sb")
                h_lo = hpool.tile([P, HK2, TB], _F8, name="hlo_sb")

                # mm1: hT[hm-tile, toks] = sum_k w1h[k, hm].T @ xT[k, toks]
                for hm in range(HK2):
                    ps = psum1.tile([P, TB], _F32, name="ps1")
                    j1, o1 = divmod(hm * P, CW)
                    cs = slice(o1, o1 + P)
                    # term-type-outer order: the first group only needs
                    # w1hi + x_hi, so the PE can start before w1lo/x_lo land
                    for i, (wt, xt) in enumerate(
                        [
                            (w1hi_sb[s][j1], xh),
                            (w1lo_sb[s][j1], xh),
                            (w1hi_sb[s][j1], xl),
                        ]
                    ):
                        for kp in range(DK // 2):
                            ks = slice(2 * kp, 2 * kp + 2)
                            nc.tensor.matmul(
                                ps[:, :tw], wt[:, ks, cs], xt[:, ks, :tw],
                                start=(i == 0 and kp == 0),
                                stop=(i == 2 and kp == DK // 2 - 1),
                                perf_mode=_DR,
                            )
                    h32 = h32pool.tile([P, TB], _F32, name="h32_sb")
                    nc.scalar.activation(
                        h32[:, :tw],
                        ps[:, :tw],
                        mybir.ActivationFunctionType.Gelu,
                        bias=b1_sb[:, s, hm : hm + 1],
                        scale=1.0 / SW,
                    )
                    nc.vector.tensor_copy(h_hi[:, hm, :tw], h32[:, :tw])
                    nc.vector.tensor_sub(
                        h_lo[:, hm, :tw], h32[:, :tw], h_hi[:, hm, :tw]
                    )

                # mm2: yT[dm-tile, toks] = sum_k w2h[k, dm].T @ hT[k, toks]
                for dm in range(DK):
                    ps2 = psum2.tile([P, TB], _F32, name="ps2")
                    j2, o2 = divmod(dm * P, CW)
                    cs = slice(o2, o2 + P)
                    for i, (wt, ht) in enumerate(
                        [
                            (w2hi_sb[s][j2], h_hi),
                            (w2lo_sb[s][j2], h_hi),
                            (w2hi_sb[s][j2], h_lo),
                        ]
                    ):
                        for kp in range(HK2 // 2):
                            ks = slice(2 * kp, 2 * kp + 2)
                            nc.tensor.matmul(
                                ps2[:, :tw], wt[:, ks, cs], ht[:, ks, :tw],
                                start=(i == 0 and kp == 0),
                                stop=(i == 2 and kp == HK2 // 2 - 1),
                                perf_mode=_DR,
                            )
                    y_sb = ypool.tile([P, TB], _BF16, name="y_sb")
                    # (ps2 * wc/SW) in one DVE op straight from PSUM
                    nc.vector.tensor_mul(
                        out=y_sb[:, :tw],
                        in0=ps2[:, :tw],
                        in1=wc_sb[:, t0 : t0 + tw],
                    )
                    nc.scalar.dma_start(
                        yT[dm * P : (dm + 1) * P, t0 : t0 + tw], y_sb[:, :tw]
                    )

    nc.compile()
    return nc


def _route(xf, gate_w, gate_b):
    """Top-2 gating in numpy. Returns per-expert (token_ids, combine_weights)."""
    gates = xf @ gate_w + gate_b  # [T, E] f32
    i1 = np.argmax(gates, axis=1)
    v1 = gates[np.arange(T), i1]
    masked = gates.copy()
    masked[np.arange(T), i1] = -np.inf
    i2 = np.argmax(masked, axis=1)
    v2 = masked[np.arange(T), i2]
    # softmax over the two top scores (v1 >= v2)
    e2 = np.exp(v2 - v1)
    g1 = 1.0 / (1.0 + e2)
    g2 = e2 / (1.0 + e2)

    tok_ids, tok_w = [], []
    for e in range(E):
        m1 = i1 == e
        m2 = i2 == e
        ids = np.concatenate([np.nonzero(m1)[0], np.nonzero(m2)[0]])
        w = np.concatenate([g1[m1], g2[m2]]).astype(np.float32)
        tok_ids.append(ids)
        tok_w.append(w)
    return tok_ids, tok_w


def _hi_lo(a):
    """Split a (already pre-scaled) f32 array into e4m3 hi + lo parts."""
    hi = a.astype(_E4)
    lo = (a - hi.astype(np.float32)).astype(_E4)
    return hi, lo


def _to_pck(a, k):
    """[K*P, N] -> chunk-major [P, N//CW, K, CW]."""
    n = a.shape[1]
    return np.ascontiguousarray(
        a.reshape(k, P, n // CW, CW).transpose(1, 2, 0, 3)
    )


def _to_pbk(a, block_starts, block_sizes):
    """[K*P, C] -> block-major [P, NB, K, TB], zero-padding blocks to TB."""
    k = a.shape[0] // P
    apk = a.reshape(k, P, -1)
    out = np.zeros((P, len(block_sizes), k, TB), dtype=a.dtype)
    for ib, (t0, tw) in enumerate(zip(block_starts, block_sizes)):
        out[:, ib, :, :tw] = apk[:, :, t0 : t0 + tw].transpose(1, 0, 2)
    return out


def kernel(x, gate_w, gate_b, w1, b1, w2, b2):
    x = np.asarray(x, dtype=np.float32)
    gate_w = np.asarray(gate_w, dtype=np.float32)
    gate_b = np.asarray(gate_b, dtype=np.float32)
    w1 = np.asarray(w1, dtype=np.float32)
    b1 = np.asarray(b1, dtype=np.float32)
    w2 = np.asarray(w2, dtype=np.float32)
    b2 = np.asarray(b2, dtype=np.float32)

    xf = x.reshape(T, D)
    tok_ids, tok_w = _route(xf, gate_w, gate_b)

    # hot/cold split: slot0 capacity = max hot load, slot1 = max cold load
    order = sorted(range(E), key=lambda e: -len(tok_ids[e]))
    hot, cold = order[:4], order[4:]
    C0 = max(TB, (max(len(tok_ids[e]) for e in hot) + 7) // 8 * 8)
    C1 = max(TB, (max(len(tok_ids[e]) for e in cold) + 7) // 8 * 8)

    key = (C0, C1)
    if key not in _RUNNER_CACHE:
        _RUNNER_CACHE[key] = _make_runner(_build(C0, C1))
    run = _RUNNER_CACHE[key]

    bs0, bz0 = _blocks(C0)
    bs1, bz1 = _blocks(C1)

    def slot_arrays(e, C):
        ids = tok_ids[e]
        cnt = len(ids)
        xe = np.zeros((C, D), dtype=np.float32)
        xe[:cnt] = xf[ids]
        wce = np.zeros((C,), dtype=np.float32)
        wce[:cnt] = tok_w[e]
        xeT_hi, xeT_lo = _hi_lo(np.ascontiguousarray(xe.T))
        return xeT_hi, xeT_lo, wce

    in_maps = []
    for i in range(4):
        a, b = hot[i], cold[i]
        xa_hi, xa_lo, wca = slot_arrays(a, C0)
        xb_hi, xb_lo, wcb = slot_arrays(b, C1)
        xhi_blocks = np.concatenate(
            [_to_pbk(xa_hi, bs0, bz0), _to_pbk(xb_hi, bs1, bz1)], axis=1
        )
        xlo_blocks = np.concatenate(
            [_to_pbk(xa_lo, bs0, bz0), _to_pbk(xb_lo, bs1, bz1)], axis=1
        )
        wc_full = np.concatenate([wca, wcb]) / SW
        for m in (0, 1):
            hs = slice(m * H2, (m + 1) * H2)
            w1a_hi, w1a_lo = _hi_lo(w1[a][:, hs] * SW)  # [D, H2]
            w1b_hi, w1b_lo = _hi_lo(w1[b][:, hs] * SW)
            w2a_hi, w2a_lo = _hi_lo(w2[a][hs, :] * SW)  # [H2, D]
            w2b_hi, w2b_lo = _hi_lo(w2[b][hs, :] * SW)
            in_maps.append(
                {
                    "xhi": xhi_blocks,
                    "xlo": xlo_blocks,
                    "w1hi": np.stack(
                        [_to_pck(w1a_hi, DK), _to_pck(w1b_hi, DK)], axis=1
                    ),
                    "w1lo": np.stack(
                        [_to_pck(w1a_lo, DK), _to_pck(w1b_lo, DK)], axis=1
                    ),
                    "w2hi": np.stack(
                        [_to_pck(w2a_hi, HK2), _to_pck(w2b_hi, HK2)], axis=1
                    ),
                    "w2lo": np.stack(
                        [_to_pck(w2a_lo, HK2), _to_pck(w2b_lo, HK2)], axis=1
                    ),
                    "b1t": np.stack(
                        [
                            np.ascontiguousarray(b1[a][hs].reshape(HK2, P).T),
                            np.ascontiguousarray(b1[b][hs].reshape(HK2, P).T),
                        ],
                        axis=1,
                    ),
                    "wc": np.broadcast_to(
                        wc_full.astype(ml_dtypes.bfloat16), (P, C0 + C1)
                    ).copy(),
                }
            )

    results = run(in_maps)

    out = np.zeros((T, D), dtype=np.float32)
    for i in range(4):
        for s, e in ((0, hot[i]), (1, cold[i])):
            ids = tok_ids[e]
            cnt = len(ids)
            c0 = 0 if s == 0 else C0
            # sum the two H-half partials (already wc-weighted on device)
            ye = (
                results[2 * i]["yT"][:, c0 : c0 + cnt].astype(np.float32)
                + results[2 * i + 1]["yT"][:, c0 : c0 + cnt].astype(np.float32)
            ).T  # [cnt, D]
            out[ids] += ye
            if b2[e].any():
                out[ids] += tok_w[e][:, None] * b2[e][None, :]
    return out.reshape(B, S, D)



# revision 2
# speedup vs baseline: 1.0065x; 1.0065x over previous
"""MoE (top-2 of 8 experts) Trainium2 kernel across 8 NeuronCores.

Host side: gating matmul + top-2 + softmax + token dispatch (part of input
sharding). Device side (per core, SPMD): TWO half-expert FFN shards. Each
expert's FFN is split into two H-halves placed on two cores; the per-core
"slot 0" runs a half of one of the 4 most-loaded (hot) experts and "slot 1"
a half of one of the 4 least-loaded (cold) experts, so the fixed SPMD token
capacities are (max hot load, max cold load) instead of (max load) twice —
near-perfect load balance. The two partial y's of an expert are summed on
the host during unsharding (b2, which is 0 here, would also be added there).

Per slot, in transposed layout:
    hT = gelu(w1h.T @ xeT + b1h)       [H/2, C_s]
    yT = (w2h.T @ hT) * wc             [D, C_s]   (partial over its H-half)

Matmuls run in fp8 (e4m3) DoubleRow perf mode with 3-term error
compensation: every operand T is split T = T_hi + T_lo (both e4m3, lo is
the quantization residual), and each logical product computes
    W_hi@x_hi + W_lo@x_hi + W_hi@x_lo
accumulated in fp32 PSUM (the dropped W_lo@x_lo term is ~0.06% relative).
This carries ~8 significant bits per operand (bf16-level accuracy) at 0.75x
the PE cost of bf16 in DoubleRow mode (2 k-tiles per instruction at 0.5
cycles/row). Weights are pre-scaled by SW=1024 so w-values sit in e4m3's
normal range; the descale is fused into the gelu activation (mm1) and the
host-prescaled combine weights (mm2).

All DRAM layouts are chunk-/block-major so every DMA moves >=512B
contiguous innermost runs (smaller elements pay a 2x DMA penalty); x blocks
are zero-padded to TB columns (the DMA moves the pad, matmuls never read
it).
"""

import numpy as np
import ml_dtypes

import jax
from jax.experimental.shard_map import shard_map
from jax.sharding import Mesh, PartitionSpec

import concourse.bass as bass  # noqa: F401
import concourse.mybir as mybir
import concourse.tile as tile
from concourse import bacc
from concourse.bass2jax import (
    _bass_exec_p,
    install_neuronx_cc_hook,
    partition_id_tensor,
)

B, S, D, H, E, TOPK = 4, 2048, 1024, 4096, 8, 2
T = B * S
P = 128
TB = 512  # max token block (matmul free dim)
SW = 1024.0  # weight pre-scale so w-values sit in e4m3 normal range
H2 = H // 2  # per-slot H half

DK = D // P  # 8  k-tiles for mm1 (contraction over D)
HK2 = H2 // P  # 16 k-tiles for mm2 (contraction over H/2), mm1 out tiles

CW = 512  # weight chunk columns

_F8 = mybir.dt.float8e4
_BF16 = mybir.dt.bfloat16
_F32 = mybir.dt.float32
_E4 = ml_dtypes.float8_e4m3
_DR = mybir.MatmulPerfMode.DoubleRow

# cache of built+compiled runners keyed by capacities, so repeat calls with
# the same shapes reuse the in-process jit executable (no re-trace/re-compile)
_RUNNER_CACHE: dict[tuple, tuple] = {}


def _make_runner(nc, n_cores=8):
    """Persistent jitted SPMD runner for a Bass module (mirrors
    concourse.bass2jax.run_bass_via_pjrt, but reusable across calls)."""
    install_neuronx_cc_hook()
    partition_name = nc.partition_id_tensor.name if nc.partition_id_tensor else None
    in_names, out_names, out_avals, zero_outs = [], [], [], []
    for alloc in nc.m.functions[0].allocations:
        if not isinstance(alloc, mybir.MemoryLocationSet):
            continue
        name = alloc.memorylocations[0].name
        if alloc.kind == "ExternalInput":
            if name != partition_name:
                in_names.append(name)
        elif alloc.kind == "ExternalOutput":
            out_names.append(name)
            shape = tuple(alloc.tensor_shape)
            dtype = mybir.dt.np(alloc.dtype)
            out_avals.append(jax.core.ShapedArray(shape, dtype))
            zero_outs.append(np.zeros(shape, dtype))
    n_params = len(in_names)
    all_in_names = list(in_names) + list(out_names)
    if partition_name is not None:
        all_in_names.append(partition_name)

    def _body(*args):
        operands = list(args)
        if partition_name is not None:
            operands.append(partition_id_tensor())
        outs = _bass_exec_p.bind(
            *operands,
            out_avals=tuple(out_avals),
            in_names=tuple(all_in_names),
            out_names=tuple(out_names),
            lowering_input_output_aliases=(),
            sim_require_finite=True,
            sim_require_nnan=True,
            nc=nc,
        )
        return tuple(outs)

    devices = jax.devices()[:n_cores]
    mesh = Mesh(np.asarray(devices), ("core",))
    n_outs = len(out_avals)
    in_specs = (PartitionSpec("core"),) * (n_params + n_outs)
    out_specs = (PartitionSpec("core"),) * n_outs
    f = jax.jit(
        shard_map(
            _body, mesh=mesh, in_specs=in_specs, out_specs=out_specs, check_rep=False
        ),
        donate_argnums=tuple(range(n_params, n_params + n_outs)),
        keep_unused=True,
    )

    def run(in_maps):
        concat_in = [
            np.concatenate([np.asarray(m[name]) for m in in_maps], axis=0)
            for name in in_names
        ]
        concat_zeros = [
            np.zeros((n_cores * z.shape[0], *z.shape[1:]), z.dtype) for z in zero_outs
        ]
        outs = f(*concat_in, *concat_zeros)
        return [
            {
                name: np.asarray(outs[i]).reshape(n_cores, *out_avals[i].shape)[c]
                for i, name in enumerate(out_names)
            }
            for c in range(n_cores)
        ]

    return run


def _blocks(C, thin_tail=0):
    """Split C columns into even blocks of at most TB (plus an optional thin
    final block, which shortens the end-of-kernel epilogue chain)."""
    if thin_tail and C > TB:
        starts, sizes = _blocks(C - thin_tail)
        return starts + [C - thin_tail], sizes + [thin_tail]
    n_blocks = (C + TB - 1) // TB
    base, rem = divmod(C, n_blocks)
    sizes = [base + (1 if i < rem else 0) for i in range(n_blocks)]
    starts = [sum(sizes[:i]) for i in range(n_blocks)]
    return starts, sizes


def _build(C0: int, C1: int):
    """Bass module: slot0 = half-FFN over C0 tokens, slot1 over C1 tokens."""
    nc = bacc.Bacc("TRN2", target_bir_lowering=False, debug=False, num_devices=8)

    NC1 = H2 // CW  # 4 w1 chunks per slot
    NC2 = D // CW  # 2 w2 chunks per slot
    caps = [C0, C1]
    col0 = [0, C0]  # column offset of each slot in wc / yT
    sblocks = [_blocks(C0), _blocks(C1, thin_tail=128)]
    NB = [len(sblocks[0][1]), len(sblocks[1][1])]
    # global block list: (slot, ib, dram_index, col_start, width)
    gblocks = []
    for s in (0, 1):
        for ib in range(NB[s]):
            gblocks.append(
                (s, ib, len(gblocks), col0[s] + sblocks[s][0][ib], sblocks[s][1][ib])
            )
    NBT = len(gblocks)
    CT = C0 + C1

    xhi = nc.dram_tensor("xhi", [P, NBT, DK, TB], _F8, kind="ExternalInput")
    xlo = nc.dram_tensor("xlo", [P, NBT, DK, TB], _F8, kind="ExternalInput")
    w1hi = nc.dram_tensor("w1hi", [P, 2, NC1, DK, CW], _F8, kind="ExternalInput")
    w1lo = nc.dram_tensor("w1lo", [P, 2, NC1, DK, CW], _F8, kind="ExternalInput")
    w2hi = nc.dram_tensor("w2hi", [P, 2, NC2, HK2, CW], _F8, kind="ExternalInput")
    w2lo = nc.dram_tensor("w2lo", [P, 2, NC2, HK2, CW], _F8, kind="ExternalInput")
    b1t = nc.dram_tensor("b1t", [P, 2, HK2], _F32, kind="ExternalInput")
    wc = nc.dram_tensor("wc", [P, CT], _BF16, kind="ExternalInput")  # pre /SW
    yT = nc.dram_tensor("yT", [D, CT], _BF16, kind="ExternalOutput")

    with tile.TileContext(nc) as tc:
        with (
            tc.tile_pool(name="wpool", bufs=1) as wpool,
            tc.tile_pool(name="xpool", bufs=2) as xpool,
            tc.tile_pool(name="hpool", bufs=2) as hpool,
            tc.tile_pool(name="h32pool", bufs=4) as h32pool,
            tc.tile_pool(name="ypool", bufs=3) as ypool,
            tc.tile_pool(name="psum1", bufs=4, space="PSUM") as psum1,
            tc.tile_pool(name="psum2", bufs=4, space="PSUM") as psum2,
        ):
            x_tiles = {}

            def load_x(g, split=False):
                xh = xpool.tile([P, DK, TB], _F8, name="xh_sb")
                xl = xpool.tile([P, DK, TB], _F8, name="xl_sb")
                if split:
                    # two half-k DMAs so the first matmul group can start
                    # after only half of x0_hi has landed
                    nc.sync.dma_start(xh[:, : DK // 2, :], xhi[:, g, : DK // 2, :])
                    nc.sync.dma_start(xh[:, DK // 2 :, :], xhi[:, g, DK // 2 :, :])
                else:
                    nc.sync.dma_start(xh[:, :, :], xhi[:, g, :, :])
                    nc.sync.dma_start(xl[:, :, :], xlo[:, g, :, :])
                x_tiles[g] = (xh, xl)
                return (xh, xl)

            w1hi_sb = [
                [wpool.tile([P, DK, CW], _F8, name=f"w1hi{s}_{j}") for j in range(NC1)]
                for s in (0, 1)
            ]
            w1lo_sb = [
                [wpool.tile([P, DK, CW], _F8, name=f"w1lo{s}_{j}") for j in range(NC1)]
                for s in (0, 1)
            ]
            w2hi_sb = [
                [wpool.tile([P, HK2, CW], _F8, name=f"w2hi{s}_{j}") for j in range(NC2)]
                for s in (0, 1)
            ]
            w2lo_sb = [
                [wpool.tile([P, HK2, CW], _F8, name=f"w2lo{s}_{j}") for j in range(NC2)]
                for s in (0, 1)
            ]
            b1_sb = wpool.tile([P, 2, HK2], _F32, name="b1_sb")
            wc_sb = wpool.tile([P, CT], _BF16, name="wc_sb")

            def load_w1(s, j):
                nc.sync.dma_start(w1hi_sb[s][j][:], w1hi[:, s, j, :, :])
                nc.sync.dma_start(w1lo_sb[s][j][:], w1lo[:, s, j, :, :])

            def load_w2(s, j):
                nc.sync.dma_start(w2hi_sb[s][j][:], w2hi[:, s, j, :, :])
                nc.sync.dma_start(w2lo_sb[s][j][:], w2lo[:, s, j, :, :])

            # startup order mirrors first consumption: the PE's first term
            # group needs only w1hi chunk0 cols 0:128 + x0_hi k0..3, the
            # second adds w1lo, the third x0_lo; b1 unblocks the first gelu
            # (which frees PSUM buffers)
            nc.sync.dma_start(w1hi_sb[0][0][:, :, 0:P], w1hi[:, 0, 0, :, 0:P])
            xh0, xl0 = load_x(0, split=True)
            nc.sync.dma_start(
                w1hi_sb[0][0][:, :, P:CW], w1hi[:, 0, 0, :, P:CW]
            )
            nc.sync.dma_start(w1lo_sb[0][0][:], w1lo[:, 0, 0, :, :])
            nc.sync.dma_start(xl0[:, : DK // 2, :], xlo[:, 0, : DK // 2, :])
            nc.sync.dma_start(xl0[:, DK // 2 :, :], xlo[:, 0, DK // 2 :, :])
            nc.sync.dma_start(b1_sb[:], b1t[:])
            for j in range(1, NC1):
                load_w1(0, j)
            nc.sync.dma_start(wc_sb[:], wc[:])
            for j in range(NC2):
                load_w2(0, j)
            if NBT > 1:
                load_x(1)
            for j in range(NC1):
                load_w1(1, j)
            for j in range(NC2):
                load_w2(1, j)

            for s, ib, g, t0, tw in gblocks:
                if g not in x_tiles:
                    load_x(g)
                xh, xl = x_tiles.pop(g)
                if g + 2 < NBT:
                    load_x(g + 2)  # blocks 0 and 1 were loaded up front

                h_hi = hpool.tile([P, HK2, TB], _F8, name="hhi_sb")
                h_lo = hpool.tile([P, HK2, TB], _F8, name="hlo_sb")

                # mm1: hT[hm-tile, toks] = sum_k w1h[k, hm].T @ xT[k, toks]
                for hm in range(HK2):
                    ps = psum1.tile([P, TB], _F32, name="ps1")
                    j1, o1 = divmod(hm * P, CW)
                    cs = slice(o1, o1 + P)
                    # term-type-outer order: the first group only needs
                    # w1hi + x_hi, so the PE can start before w1lo/x_lo land
                    for i, (wt, xt) in enumerate(
                        [
                            (w1hi_sb[s][j1], xh),
                            (w1lo_sb[s][j1], xh),
                            (w1hi_sb[s][j1], xl),
                        ]
                    ):
                        for kp in range(DK // 2):
                            ks = slice(2 * kp, 2 * kp + 2)
                            nc.tensor.matmul(
                                ps[:, :tw], wt[:, ks, cs], xt[:, ks, :tw],
                                start=(i == 0 and kp == 0),
                                stop=(i == 2 and kp == DK // 2 - 1),
                                perf_mode=_DR,
                            )
                    h32 = h32pool.tile([P, TB], _F32, name="h32_sb")
                    nc.scalar.activation(
                        h32[:, :tw],
                        ps[:, :tw],
                        mybir.ActivationFunctionType.Gelu,
                        bias=b1_sb[:, s, hm : hm + 1],
                        scale=1.0 / SW,
                    )
                    nc.vector.tensor_copy(h_hi[:, hm, :tw], h32[:, :tw])
                    nc.vector.tensor_sub(
                        h_lo[:, hm, :tw], h32[:, :tw], h_hi[:, hm, :tw]
                    )

                # mm2: yT[dm-tile, toks] = sum_k w2h[k, dm].T @ hT[k, toks]
                for dm in range(DK):
                    ps2 = psum2.tile([P, TB], _F32, name="ps2")
                    j2, o2 = divmod(dm * P, CW)
                    cs = slice(o2, o2 + P)
                    for i, (wt, ht) in enumerate(
                        [
                            (w2hi_sb[s][j2], h_hi),
                            (w2lo_sb[s][j2], h_hi),
                            (w2hi_sb[s][j2], h_lo),
                        ]
                    ):
                        for kp in range(HK2 // 2):
                            ks = slice(2 * kp, 2 * kp + 2)
                            nc.tensor.matmul(
                                ps2[:, :tw], wt[:, ks, cs], ht[:, ks, :tw],
                                start=(i == 0 and kp == 0),
                                stop=(i == 2 and kp == HK2 // 2 - 1),
                                perf_mode=_DR,
                            )
                    y_sb = ypool.tile([P, TB], _BF16, name="y_sb")
                    # (ps2 * wc/SW) in one DVE op straight from PSUM
                    nc.vector.tensor_mul(
                        out=y_sb[:, :tw],
                        in0=ps2[:, :tw],
                        in1=wc_sb[:, t0 : t0 + tw],
                    )
                    nc.scalar.dma_start(
                        yT[dm * P : (dm + 1) * P, t0 : t0 + tw], y_sb[:, :tw]
                    )

    nc.compile()
    return nc


def _route(xf, gate_w, gate_b):
    """Top-2 gating in numpy. Returns per-expert (token_ids, combine_weights)."""
    gates = xf @ gate_w + gate_b  # [T, E] f32
    i1 = np.argmax(gates, axis=1)
    v1 = gates[np.arange(T), i1]
    masked = gates.copy()
    masked[np.arange(T), i1] = -np.inf
    i2 = np.argmax(masked, axis=1)
    v2 = masked[np.arange(T), i2]
    # softmax over the two top scores (v1 >= v2)
    e2 = np.exp(v2 - v1)
    g1 = 1.0 / (1.0 + e2)
    g2 = e2 / (1.0 + e2)

    tok_ids, tok_w = [], []
    for e in range(E):
        m1 = i1 == e
        m2 = i2 == e
        ids = np.concatenate([np.nonzero(m1)[0], np.nonzero(m2)[0]])
        w = np.concatenate([g1[m1], g2[m2]]).astype(np.float32)
        tok_ids.append(ids)
        tok_w.append(w)
    return tok_ids, tok_w


def _hi_lo(a):
    """Split a (already pre-scaled) f32 array into e4m3 hi + lo parts."""
    hi = a.astype(_E4)
    lo = (a - hi.astype(np.float32)).astype(_E4)
    return hi, lo


def _to_pck(a, k):
    """[K*P, N] -> chunk-major [P, N//CW, K, CW]."""
    n = a.shape[1]
    return np.ascontiguousarray(
        a.reshape(k, P, n // CW, CW).transpose(1, 2, 0, 3)
    )


def _to_pbk(a, block_starts, block_sizes):
    """[K*P, C] -> block-major [P, NB, K, TB], zero-padding blocks to TB."""
    k = a.shape[0] // P
    apk = a.reshape(k, P, -1)
    out = np.zeros((P, len(block_sizes), k, TB), dtype=a.dtype)
    for ib, (t0, tw) in enumerate(zip(block_starts, block_sizes)):
        out[:, ib, :, :tw] = apk[:, :, t0 : t0 + tw].transpose(1, 0, 2)
    return out


def kernel(x, gate_w, gate_b, w1, b1, w2, b2):
    x = np.asarray(x, dtype=np.float32)
    gate_w = np.asarray(gate_w, dtype=np.float32)
    gate_b = np.asarray(gate_b, dtype=np.float32)
    w1 = np.asarray(w1, dtype=np.float32)
    b1 = np.asarray(b1, dtype=np.float32)
    w2 = np.asarray(w2, dtype=np.float32)
    b2 = np.asarray(b2, dtype=np.float32)

    xf = x.reshape(T, D)
    tok_ids, tok_w = _route(xf, gate_w, gate_b)

    # hot/cold split: slot0 capacity = max hot load, slot1 = max cold load
    order = sorted(range(E), key=lambda e: -len(tok_ids[e]))
    hot, cold = order[:4], order[4:]
    C0 = max(TB, (max(len(tok_ids[e]) for e in hot) + 7) // 8 * 8)
    C1 = max(TB, (max(len(tok_ids[e]) for e in cold) + 7) // 8 * 8)

    key = (C0, C1)
    if key not in _RUNNER_CACHE:
        _RUNNER_CACHE[key] = _make_runner(_build(C0, C1))
    run = _RUNNER_CACHE[key]

    bs0, bz0 = _blocks(C0)
    bs1, bz1 = _blocks(C1, thin_tail=128)

    def slot_arrays(e, C):
        ids = tok_ids[e]
        cnt = len(ids)
        xe = np.zeros((C, D), dtype=np.float32)
        xe[:cnt] = xf[ids]
        wce = np.zeros((C,), dtype=np.float32)
        wce[:cnt] = tok_w[e]
        xeT_hi, xeT_lo = _hi_lo(np.ascontiguousarray(xe.T))
        return xeT_hi, xeT_lo, wce

    in_maps = []
    for i in range(4):
        a, b = hot[i], cold[i]
        xa_hi, xa_lo, wca = slot_arrays(a, C0)
        xb_hi, xb_lo, wcb = slot_arrays(b, C1)
        xhi_blocks = np.concatenate(
            [_to_pbk(xa_hi, bs0, bz0), _to_pbk(xb_hi, bs1, bz1)], axis=1
        )
        xlo_blocks = np.concatenate(
            [_to_pbk(xa_lo, bs0, bz0), _to_pbk(xb_lo, bs1, bz1)], axis=1
        )
        wc_full = np.concatenate([wca, wcb]) / SW
        for m in (0, 1):
            hs = slice(m * H2, (m + 1) * H2)
            w1a_hi, w1a_lo = _hi_lo(w1[a][:, hs] * SW)  # [D, H2]
            w1b_hi, w1b_lo = _hi_lo(w1[b][:, hs] * SW)
            w2a_hi, w2a_lo = _hi_lo(w2[a][hs, :] * SW)  # [H2, D]
            w2b_hi, w2b_lo = _hi_lo(w2[b][hs, :] * SW)
            in_maps.append(
                {
                    "xhi": xhi_blocks,
                    "xlo": xlo_blocks,
                    "w1hi": np.stack(
                        [_to_pck(w1a_hi, DK), _to_pck(w1b_hi, DK)], axis=1
                    ),
                    "w1lo": np.stack(
                        [_to_pck(w1a_lo, DK), _to_pck(w1b_lo, DK)], axis=1
                    ),
                    "w2hi": np.stack(
                        [_to_pck(w2a_hi, HK2), _to_pck(w2b_hi, HK2)], axis=1
                    ),
                    "w2lo": np.stack(
                        [_to_pck(w2a_lo, HK2), _to_pck(w2b_lo, HK2)], axis=1
                    ),
                    "b1t": np.stack(
                        [
                            np.ascontiguousarray(b1[a][hs].reshape(HK2, P).T),
                            np.ascontiguousarray(b1[b][hs].reshape(HK2, P).T),
                        ],
                        axis=1,
                    ),
                    "wc": np.broadcast_to(
                        wc_full.astype(ml_dtypes.bfloat16), (P, C0 + C1)
                    ).copy(),
                }
            )

    results = run(in_maps)

    out = np.zeros((T, D), dtype=np.float32)
    for i in range(4):
        for s, e in ((0, hot[i]), (1, cold[i])):
            ids = tok_ids[e]
            cnt = len(ids)
            c0 = 0 if s == 0 else C0
            # sum the two H-half partials (already wc-weighted on device)
            ye = (
                results[2 * i]["yT"][:, c0 : c0 + cnt].astype(np.float32)
                + results[2 * i + 1]["yT"][:, c0 : c0 + cnt].astype(np.float32)
            ).T  # [cnt, D]
            out[ids] += ye
            if b2[e].any():
                out[ids] += tok_w[e][:, None] * b2[e][None, :]
    return out.reshape(B, S, D)

